# revision 11
# baseline (speedup 1.0000x reference)
"""KGCompletionGNN Trainium2 kernel v3 (8 NeuronCores, SPMD edge-sharding).

bf16 datapath + message AllToAll aggregation; validated primitives only:
  - indirect DMA: [128,1] offsets, [128,128] tiles (gather + scatter)
  - direct HWDGE slab loads for phase B (sender-placed interleaved layout)
Per layer: phase A computes edge update + messages, scatters messages into
an AllToAll buffer laid out so the receiver reads plain 1024-row slabs;
AllToAll; phase B aggregates with one-hot matmuls straight into the H
update (SBUF-resident residual); bf16 AllGather between layers.
"""

import sys

sys.path.insert(0, "/opt/trn_rl_repo")

import numpy as np
import ml_dtypes

BF = ml_dtypes.bfloat16
P = 128
SC = 2048           # edges per superchunk (et/en streaming only)
NSUB = SC // P      # 16
SLAB = 1024         # a2a rows per slab = 8 tiles
LRELU_SLOPE = 0.01
LN_EPS = 1e-5


# ---------------------------------------------------------------- host prep
def _prep_host(H, E, ht, params, ncores):
    n, d = H.shape
    m = E.shape[0]
    assert d == P
    n_pad = -(-n // (ncores * P)) * (ncores * P)
    shard_n = n_pad // ncores
    shard_blocks = shard_n // P
    nblocks = n_pad // P
    m_loc = m // ncores
    scn = -(-m_loc // SC)
    m_pad = scn * SC

    meta = dict(n=n, d=d, m=m, n_pad=n_pad, shard_n=shard_n,
                shard_blocks=shard_blocks, m_loc=m_loc, m_pad=m_pad, scn=scn,
                ncores=ncores, L=params["W_eu"].shape[0])

    flags = dict(
        beu=bool(np.any(params["b_eu"])), bf=bool(np.any(params["b_fwd"])),
        bb=bool(np.any(params["b_back"])),
        ge=bool(np.any(params["ln_e_g"] != 1)), be=bool(np.any(params["ln_e_b"])),
        gh=bool(np.any(params["ln_h_g"] != 1)), bh=bool(np.any(params["ln_h_b"])),
    )
    meta["flags"] = flags

    H_pad = np.zeros((n_pad, d), np.float32)
    H_pad[:n] = H

    cnt = (np.bincount(ht[:, 1], minlength=n_pad)
           + np.bincount(ht[:, 0], minlength=n_pad)).astype(np.float32)
    inv_cnt = (1.0 / np.maximum(cnt, 1.0)).astype(np.float32)

    def slot_table(arr):
        # arr [m_pad] -> [128, scn*NSUB]; [p, sc*NSUB+t] = arr[sc*SC+p*NSUB+t]
        return np.ascontiguousarray(
            arr.reshape(scn, P, NSUB).transpose(1, 0, 2).reshape(P, scn * NSUB))

    heads, tails, reals = [], [], []
    for c in range(ncores):
        sl = slice(c * m_loc, (c + 1) * m_loc)
        h_s = np.zeros(m_pad, np.int64)
        t_s = np.zeros(m_pad, np.int64)
        h_s[:m_loc] = ht[sl, 0]
        t_s[:m_loc] = ht[sl, 1]
        real = np.zeros(m_pad, bool)
        real[:m_loc] = True
        heads.append(h_s); tails.append(t_s); reals.append(real)

    # sorted message streams per core (fwd=slots, back=slots; dst block g)
    g_sorted, rel_sorted, order_list, Lg = [], [], [], []
    for c in range(ncores):
        dst = np.concatenate([tails[c], heads[c]])
        real2 = np.concatenate([reals[c], reals[c]])
        g = (dst >> 7).astype(np.int64)
        order = np.argsort(g, kind="stable")
        order = order[real2[order]]
        gs = g[order]
        g_sorted.append(gs)
        rel_sorted.append((dst[order] & 127).astype(np.int64))
        order_list.append(order)
        Lg.append(np.bincount(gs, minlength=nblocks))

    # shared per-(sender, local block) tile counts
    nt_sh = np.zeros((ncores, shard_blocks), np.int64)
    for s in range(ncores):
        Ls = Lg[s].reshape(ncores, shard_blocks)   # [r, j]
        nt_sh[s] = (-(-Ls // P)).max(axis=0)
    B_base = np.zeros((ncores, shard_blocks + 1), np.int64)
    B_base[:, 1:] = np.cumsum(nt_sh, axis=1)
    JS = shard_blocks // 2
    S_tA = int(-(-B_base[:, JS].max() // 8) * 8)
    S_tB = int(-(-(B_base[:, -1] - B_base[:, JS]).max() // 8) * 8)
    REGA = S_tA * P
    REGB = S_tB * P
    meta["JS"] = JS
    meta["REGA"] = REGA
    meta["REGB"] = REGB
    meta["S_tA"] = S_tA
    meta["S_tB"] = S_tB
    k_b = nt_sh.sum(axis=0)                        # [j]
    colbase = np.zeros((shard_blocks, ncores + 1), np.int64)
    colbase[:, 1:] = np.cumsum(nt_sh.T, axis=1)    # within block j, per sender
    jbase = np.zeros(shard_blocks + 1, np.int64)
    jbase[1:] = np.cumsum(k_b)
    T = int(jbase[-1])
    meta["k_b"] = k_b
    meta["T"] = T
    meta["nt_sh"] = nt_sh
    meta["B_base"] = B_base

    # within-region group offsets in the sorted stream (per sender)
    start_r = np.zeros((ncores, ncores + 1), np.int64)
    for c in range(ncores):
        start_r[c, 1:] = np.cumsum(Lg[c].reshape(ncores, shard_blocks).sum(axis=1))

    # scatter position tables (half A rows first, half B after 8*REGA)
    fpos_t, bpos_t = [], []
    for c in range(ncores):
        gs = g_sorted[c]
        starts = np.searchsorted(gs, np.arange(nblocks))
        pos_in_group = np.arange(len(gs)) - starts[gs]
        r_of = gs // shard_blocks
        j_of = gs % shard_blocks
        tau = B_base[c][j_of] + pos_in_group // P
        pp = pos_in_group % P
        tauB = tau - B_base[c][JS]
        rowsA = r_of * REGA + (tau // 8) * SLAB + 8 * pp + (tau % 8)
        rowsB = (8 * REGA + r_of * REGB + (tauB // 8) * SLAB + 8 * pp
                 + (tauB % 8))
        rows = np.where(j_of < JS, rowsA, rowsB)
        pos_of_msg = np.full(2 * m_pad, 2**30, np.int64)
        pos_of_msg[order_list[c]] = rows
        fpos_t.append(slot_table(pos_of_msg[:m_pad].astype(np.int32)))
        bpos_t.append(slot_table(pos_of_msg[m_pad:].astype(np.int32)))

    # receiver rel tables [128, T]
    relv_t = []
    for r in range(ncores):
        relv = np.full((P, T), 999.0, np.float32)
        for j in range(shard_blocks):
            for s in range(ncores):
                g = r * shard_blocks + j
                L = int(Lg[s][g])
                if L == 0:
                    continue
                so = int(start_r[s][r]
                         + (Lg[s][r * shard_blocks:g]).sum())
                rels = rel_sorted[s][so:so + L]
                for q in range(int(nt_sh[s][j])):
                    col = int(jbase[j] + colbase[j][s] + q)
                    i0 = q * P
                    ncnt = max(0, min(P, L - i0))
                    if ncnt > 0:
                        relv[:ncnt, col] = rels[i0:i0 + ncnt]
        relv_t.append(relv)

    iota = np.broadcast_to(np.arange(P, dtype=np.float32), (P, P)).astype(BF).copy()
    per_core = []
    for c in range(ncores):
        sl = slice(c * m_loc, (c + 1) * m_loc)
        E_pad = np.zeros((m_pad, d), np.float32)
        E_pad[:m_loc] = E[sl]
        pc = dict(
            h_in=np.ascontiguousarray(H_pad.astype(BF)),
            e_in=np.ascontiguousarray(E_pad.astype(BF).reshape(scn, P, SC)),
            hidx=slot_table(heads[c].astype(np.int32)),
            tidx=slot_table(tails[c].astype(np.int32)),
            fpos=fpos_t[c], bpos=bpos_t[c],
            relv=relv_t[c],
            invc=np.ascontiguousarray(
                inv_cnt[c * shard_n:(c + 1) * shard_n].reshape(shard_blocks, P).T),
            h_res=np.ascontiguousarray(H_pad[c * shard_n:(c + 1) * shard_n]),
            iota=iota,
        )
        L = meta["L"]
        for l in range(L):
            for j in range(3):
                pc[f"weu{j}_{l}"] = np.ascontiguousarray(
                    params["W_eu"][l][j * P:(j + 1) * P, :].astype(BF))
            for j in range(2):
                pc[f"wf{j}_{l}"] = np.ascontiguousarray(
                    params["W_fwd"][l][j * P:(j + 1) * P, :].astype(BF))
                pc[f"wb{j}_{l}"] = np.ascontiguousarray(
                    params["W_back"][l][j * P:(j + 1) * P, :].astype(BF))
            for nm, key, flag in [
                ("beu", "b_eu", flags["beu"]), ("bf", "b_fwd", flags["bf"]),
                ("bb", "b_back", flags["bb"]), ("ge", "ln_e_g", flags["ge"]),
                ("be", "ln_e_b", flags["be"]), ("gh", "ln_h_g", flags["gh"]),
                ("bh", "ln_h_b", flags["bh"]),
            ]:
                if flag:
                    pc[f"{nm}_{l}"] = np.broadcast_to(
                        params[key][l], (P, d)).astype(np.float32).copy()
        per_core.append(pc)
    return meta, per_core


# ---------------------------------------------------------------- program
def _build_program(meta):
    import concourse.bacc as bacc
    import concourse.tile as tile
    from concourse import bass, mybir
    from concourse.bass import IndirectOffsetOnAxis
    from concourse.masks import make_identity

    f32 = mybir.dt.float32
    bf16 = mybir.dt.bfloat16
    i32 = mybir.dt.int32
    Alu = mybir.AluOpType
    Act = mybir.ActivationFunctionType

    d = meta["d"]
    L = meta["L"]
    fl = meta["flags"]
    scn = meta["scn"]
    REGA = meta["REGA"]
    REGB = meta["REGB"]
    JS = meta["JS"]
    T = meta["T"]
    k_b = meta["k_b"]
    shard_blocks = meta["shard_blocks"]
    shard_n = meta["shard_n"]
    n_pad = meta["n_pad"]
    ncores = meta["ncores"]
    rg = [list(range(ncores))]
    ntab = scn * NSUB
    NROWS = ncores * (REGA + REGB)
    OFFB = ncores * REGA

    nc = bacc.Bacc("TRN2", target_bir_lowering=False)

    h_in = nc.dram_tensor("h_in", [n_pad, d], bf16, kind="ExternalInput")
    e_in = nc.dram_tensor("e_in", [scn, P, SC], bf16, kind="ExternalInput")
    hidx = nc.dram_tensor("hidx", [P, ntab], i32, kind="ExternalInput")
    tidx = nc.dram_tensor("tidx", [P, ntab], i32, kind="ExternalInput")
    fpos = nc.dram_tensor("fpos", [P, ntab], i32, kind="ExternalInput")
    bpos = nc.dram_tensor("bpos", [P, ntab], i32, kind="ExternalInput")
    relv = nc.dram_tensor("relv", [P, T], f32, kind="ExternalInput")
    invc = nc.dram_tensor("invc", [P, shard_blocks], f32, kind="ExternalInput")
    h_res_in = nc.dram_tensor("h_res", [shard_n, d], f32, kind="ExternalInput")
    iota_in = nc.dram_tensor("iota", [P, P], bf16, kind="ExternalInput")
    h_out = nc.dram_tensor("h_out", [shard_n, d], f32, kind="ExternalOutput")

    win = {}
    for l in range(L):
        for j in range(3):
            win[f"weu{j}_{l}"] = nc.dram_tensor(f"weu{j}_{l}", [P, d], bf16,
                                                kind="ExternalInput")
        for j in range(2):
            win[f"wf{j}_{l}"] = nc.dram_tensor(f"wf{j}_{l}", [P, d], bf16,
                                               kind="ExternalInput")
            win[f"wb{j}_{l}"] = nc.dram_tensor(f"wb{j}_{l}", [P, d], bf16,
                                               kind="ExternalInput")
        for nm, flag in [("beu", fl["beu"]), ("bf", fl["bf"]), ("bb", fl["bb"]),
                         ("ge", fl["ge"]), ("be", fl["be"]),
                         ("gh", fl["gh"]), ("bh", fl["bh"])]:
            if flag:
                win[f"{nm}_{l}"] = nc.dram_tensor(f"{nm}_{l}", [P, d], f32,
                                                  kind="ExternalInput")

    with tile.TileContext(nc) as tc:
        with (
            tc.tile_pool(name="const", bufs=1) as cp,
            tc.tile_pool(name="dram", bufs=1, space="DRAM") as dp,
            tc.tile_pool(name="sb", bufs=3) as sp,
            tc.tile_pool(name="big", bufs=2) as bp,
            tc.tile_pool(name="slabs", bufs=3) as lp,
            tc.tile_pool(name="sbsmall", bufs=4) as ssp,
            tc.tile_pool(name="ps", bufs=2, space="PSUM") as pp,
            tc.tile_pool(name="pst", bufs=2, space="PSUM") as pt,
        ):
            e_mid = dp.tile([scn, P, SC], bf16, tag="e_mid")
            a2a_in = dp.tile([NROWS, d], bf16, tag="a2a_in", name="a2a_in")
            a2a_outA = dp.tile([ncores * REGA, d], bf16, tag="a2a_outA",
                               name="a2a_outA")
            a2a_outB = dp.tile([ncores * REGB, d], bf16, tag="a2a_outB",
                               name="a2a_outB")
            h_new_bf = dp.tile([shard_n, d], bf16, tag="h_new_bf")
            h_full_bf = dp.tile([n_pad, d], bf16, tag="h_full_bf",
                                addr_space="Shared")

            ident_b = cp.tile([P, P], bf16, tag="ident_b")
            make_identity(nc, ident_b[:])
            eps_t = cp.tile([P, 1], f32, tag="eps")
            nc.vector.memset(eps_t[:], LN_EPS)
            iota_t = cp.tile([P, P], bf16, tag="iota")
            nc.sync.dma_start(out=iota_t[:], in_=iota_in[:])
            hidx_t = cp.tile([P, ntab], i32, tag="hidx")
            nc.sync.dma_start(out=hidx_t[:], in_=hidx[:])
            tidx_t = cp.tile([P, ntab], i32, tag="tidx")
            nc.sync.dma_start(out=tidx_t[:], in_=tidx[:])
            fpos_t = cp.tile([P, ntab], i32, tag="fpos")
            nc.sync.dma_start(out=fpos_t[:], in_=fpos[:])
            bpos_t = cp.tile([P, ntab], i32, tag="bpos")
            nc.sync.dma_start(out=bpos_t[:], in_=bpos[:])
            relv_t = cp.tile([P, T], f32, tag="relv")
            nc.sync.dma_start(out=relv_t[:], in_=relv[:])
            invc_t = cp.tile([P, shard_blocks], f32, tag="invc")
            nc.sync.dma_start(out=invc_t[:], in_=invc[:])
            h_res = cp.tile([P, shard_n], f32, tag="h_res")
            for j in range(shard_blocks):
                nc.sync.dma_start(out=h_res[:, j * P:(j + 1) * P],
                                  in_=h_res_in[j * P:(j + 1) * P, :])
            zero_t = cp.tile([P, SLAB], bf16, tag="zero")
            nc.vector.memset(zero_t[:], 0.0)
            for w in range(NROWS // SLAB):
                nc.sync.dma_start(out=a2a_in[w * SLAB:(w + 1) * SLAB, :],
                                  in_=zero_t[:])

            wt = {}
            for key, t_in in win.items():
                t = cp.tile([P, d], t_in.dtype, tag=key)
                nc.sync.dma_start(out=t[:], in_=t_in[:])
                wt[key] = t

            for l in range(L):
                h_src = h_in if l == 0 else h_full_bf
                e_src = e_in if l == 0 else e_mid

                # ================= phase A (gathers pipelined one sc ahead)
                def issue_gathers(sc_i):
                    c0 = sc_i * NSUB
                    xs, ts = [], []
                    for t in range(NSUB):
                        col = c0 + t
                        xh = sp.tile([P, P], bf16, tag="xh", bufs=52)
                        nc.gpsimd.indirect_dma_start(
                            out=xh[:], out_offset=None, in_=h_src[:],
                            in_offset=IndirectOffsetOnAxis(
                                ap=hidx_t[:, col:col + 1], axis=0))
                        xt = sp.tile([P, P], bf16, tag="xt", bufs=52)
                        nc.gpsimd.indirect_dma_start(
                            out=xt[:], out_offset=None, in_=h_src[:],
                            in_offset=IndirectOffsetOnAxis(
                                ap=tidx_t[:, col:col + 1], axis=0))
                        xs.append(xh); ts.append(xt)
                    return xs, ts

                pend = [issue_gathers(0)]
                if scn > 1:
                    pend.append(issue_gathers(1))
                for sc_i in range(scn):
                    c0 = sc_i * NSUB
                    et4 = bp.tile([P, SC], bf16, tag="et4")
                    nc.sync.dma_start(out=et4[:], in_=e_src[sc_i])
                    en4 = bp.tile([P, SC], bf16, tag="en4")
                    xh_list, xt_list = pend.pop(0)
                    if sc_i + 2 < scn:
                        pend.append(issue_gathers(sc_i + 2))

                    for t in range(NSUB):
                        col = c0 + t
                        slc = slice(t * P, (t + 1) * P)
                        xh = xh_list[t]
                        xt = xt_list[t]

                        def transp(src_ap, tag, eng):
                            ps = pt.tile([P, P], bf16, tag="trps")
                            nc.tensor.transpose(out=ps[:], in_=src_ap,
                                                identity=ident_b[:])
                            tt = sp.tile([P, P], bf16, tag=tag)
                            if eng == "act":
                                nc.scalar.copy(tt[:], ps[:])
                            else:
                                nc.vector.tensor_copy(out=tt[:], in_=ps[:])
                            return tt

                        xhT = transp(xh[:], "xhT", "act")
                        xtT = transp(xt[:], "xtT", "dve")
                        etT = transp(et4[:, slc], "etT", "act")

                        eu = pp.tile([P, d], f32, tag="eu")
                        nc.tensor.matmul(out=eu[:], lhsT=xhT[:],
                                         rhs=wt[f"weu0_{l}"][:], start=True,
                                         stop=False)
                        nc.tensor.matmul(out=eu[:], lhsT=etT[:],
                                         rhs=wt[f"weu1_{l}"][:], start=False,
                                         stop=False)
                        nc.tensor.matmul(out=eu[:], lhsT=xtT[:],
                                         rhs=wt[f"weu2_{l}"][:], start=False,
                                         stop=True)

                        if fl["beu"]:
                            eub = sp.tile([P, d], f32, tag="eub")
                            nc.vector.tensor_add(eub[:], eu[:], wt[f"beu_{l}"][:])
                            eusrc = eub
                        else:
                            eusrc = eu
                        t01 = sp.tile([P, d], f32, tag="t01")
                        nc.scalar.activation(t01[:], eusrc[:], Act.Identity,
                                             scale=LRELU_SLOPE)
                        z3 = sp.tile([P, d], f32, tag="z3")
                        nc.vector.tensor_tensor(out=z3[:], in0=eusrc[:],
                                                in1=t01[:], op=Alu.max)
                        z2 = sp.tile([P, d], f32, tag="z2")
                        nc.vector.tensor_add(z2[:], z3[:], et4[:, slc])

                        st6 = ssp.tile([P, 6], f32, tag="st6e")
                        nc.vector.bn_stats(st6[:], z2[:])
                        st2 = ssp.tile([P, 2], f32, tag="st2e")
                        nc.vector.bn_aggr(st2[:], st6[:])
                        std = ssp.tile([P, 1], f32, tag="stde")
                        nc.scalar.activation(std[:], st2[:, 1:2], Act.Sqrt,
                                             bias=eps_t[:, 0:1])
                        istd = ssp.tile([P, 1], f32, tag="istde")
                        nc.vector.reciprocal(istd[:], std[:])
                        nmu = ssp.tile([P, 1], f32, tag="nmue")
                        nc.vector.tensor_scalar(nmu[:], st2[:, 0:1], istd[:, 0:1],
                                                -1.0, Alu.mult, Alu.mult)
                        if fl["ge"] or fl["be"]:
                            enf = sp.tile([P, d], f32, tag="enf")
                            nc.scalar.activation(enf[:], z2[:], Act.Identity,
                                                 bias=nmu[:, 0:1],
                                                 scale=istd[:, 0:1])
                            if fl["ge"]:
                                nc.vector.tensor_mul(enf[:], enf[:], wt[f"ge_{l}"][:])
                            if fl["be"]:
                                nc.vector.tensor_add(enf[:], enf[:], wt[f"be_{l}"][:])
                            nc.vector.tensor_copy(out=en4[:, slc], in_=enf[:])
                        else:
                            nc.scalar.activation(en4[:, slc], z2[:], Act.Identity,
                                                 bias=nmu[:, 0:1], scale=istd[:, 0:1])

                        enT = transp(en4[:, slc], "enT", "dve")

                        mf = pp.tile([P, d], f32, tag="mm")
                        nc.tensor.matmul(out=mf[:], lhsT=xhT[:],
                                         rhs=wt[f"wf0_{l}"][:], start=True,
                                         stop=False)
                        nc.tensor.matmul(out=mf[:], lhsT=enT[:],
                                         rhs=wt[f"wf1_{l}"][:], start=False,
                                         stop=True)
                        mfs = sp.tile([P, P], bf16, tag="mfs")
                        if fl["bf"]:
                            mff = sp.tile([P, d], f32, tag="mff")
                            nc.vector.tensor_add(mff[:], mf[:], wt[f"bf_{l}"][:])
                            nc.vector.tensor_copy(out=mfs[:], in_=mff[:])
                        else:
                            nc.scalar.copy(mfs[:], mf[:])
                        nc.gpsimd.indirect_dma_start(
                            out=a2a_in[:],
                            out_offset=IndirectOffsetOnAxis(
                                ap=fpos_t[:, col:col + 1], axis=0),
                            in_=mfs[:], in_offset=None,
                            bounds_check=NROWS - 1, oob_is_err=False)

                        mb = pp.tile([P, d], f32, tag="mm")
                        nc.tensor.matmul(out=mb[:], lhsT=xtT[:],
                                         rhs=wt[f"wb0_{l}"][:], start=True,
                                         stop=False)
                        nc.tensor.matmul(out=mb[:], lhsT=enT[:],
                                         rhs=wt[f"wb1_{l}"][:], start=False,
                                         stop=True)
                        mbs = sp.tile([P, P], bf16, tag="mbs")
                        if fl["bb"]:
                            mbf = sp.tile([P, d], f32, tag="mbf")
                            nc.vector.tensor_add(mbf[:], mb[:], wt[f"bb_{l}"][:])
                            nc.vector.tensor_copy(out=mbs[:], in_=mbf[:])
                        else:
                            nc.scalar.copy(mbs[:], mb[:])
                        nc.gpsimd.indirect_dma_start(
                            out=a2a_in[:],
                            out_offset=IndirectOffsetOnAxis(
                                ap=bpos_t[:, col:col + 1], axis=0),
                            in_=mbs[:], in_offset=None,
                            bounds_check=NROWS - 1, oob_is_err=False)

                    if l == 0:
                        nc.sync.dma_start(out=e_mid[sc_i], in_=en4[:])

                # ================= AllToAll (split halves for overlap)
                nc.gpsimd.collective_compute(
                    "AllToAll", mybir.AluOpType.bypass, replica_groups=rg,
                    ins=[a2a_in[0:OFFB, :]], outs=[a2a_outA[:]])
                nc.gpsimd.collective_compute(
                    "AllToAll", mybir.AluOpType.bypass, replica_groups=rg,
                    ins=[a2a_in[OFFB:NROWS, :]], outs=[a2a_outB[:]])

                # ================= phase B + H update
                # lazy slab loads per source region
                slab_tiles = [None] * ncores
                slab_idx = [-1] * ncores
                col = 0
                for j in range(shard_blocks):
                    agg = pp.tile([P, d], f32, tag="agg")
                    ktot = int(k_b[j])
                    kk = 0
                    for s in range(ncores):
                        ntq = int(meta["nt_sh"][s][j])
                        for q in range(ntq):
                            if j < JS:
                                tau = int(meta["B_base"][s][j] + q)
                                srcten, reg = a2a_outA, REGA
                            else:
                                tau = int(meta["B_base"][s][j]
                                          - meta["B_base"][s][JS] + q)
                                srcten, reg = a2a_outB, REGB
                            w = tau // 8
                            sub = tau % 8
                            key = (j < JS, w)
                            if slab_idx[s] != key:
                                st = lp.tile([P, SLAB], bf16, tag=f"slab{s}")
                                base = s * reg + w * SLAB
                                nc.sync.dma_start(
                                    out=st[:],
                                    in_=srcten[base:base + SLAB, :])
                                slab_tiles[s] = st
                                slab_idx[s] = key
                            oh = sp.tile([P, P], bf16, tag="oh")
                            nc.vector.tensor_scalar(oh[:], iota_t[:],
                                                    relv_t[:, col:col + 1],
                                                    None, Alu.is_equal)
                            nc.tensor.matmul(
                                out=agg[:], lhsT=oh[:],
                                rhs=slab_tiles[s][:, sub * P:(sub + 1) * P],
                                start=(kk == 0), stop=(kk == ktot - 1))
                            kk += 1
                            col += 1
                    assert kk == ktot

                    jsl = slice(j * P, (j + 1) * P)
                    mn = sp.tile([P, d], f32, tag="mn")
                    nc.vector.tensor_scalar(mn[:], agg[:], invc_t[:, j:j + 1],
                                            None, Alu.mult)
                    t01h = sp.tile([P, d], f32, tag="t01h")
                    nc.scalar.activation(t01h[:], mn[:], Act.Identity,
                                         scale=LRELU_SLOPE)
                    zh = sp.tile([P, d], f32, tag="zh")
                    nc.vector.tensor_tensor(out=zh[:], in0=mn[:], in1=t01h[:],
                                            op=Alu.max)
                    z2h = sp.tile([P, d], f32, tag="z2h")
                    nc.vector.tensor_add(z2h[:], zh[:], h_res[:, jsl])
                    st6h = ssp.tile([P, 6], f32, tag="st6h")
                    nc.vector.bn_stats(st6h[:], z2h[:])
                    st2h = ssp.tile([P, 2], f32, tag="st2h")
                    nc.vector.bn_aggr(st2h[:], st6h[:])
                    stdh = ssp.tile([P, 1], f32, tag="stdh")
                    nc.scalar.activation(stdh[:], st2h[:, 1:2], Act.Sqrt,
                                         bias=eps_t[:, 0:1])
                    istdh = ssp.tile([P, 1], f32, tag="istdh")
                    nc.vector.reciprocal(istdh[:], stdh[:])
                    nmuh = ssp.tile([P, 1], f32, tag="nmuh")
                    nc.vector.tensor_scalar(nmuh[:], st2h[:, 0:1], istdh[:, 0:1],
                                            -1.0, Alu.mult, Alu.mult)
                    gb = fl["gh"] or fl["bh"]
                    if gb:
                        hnf = sp.tile([P, d], f32, tag="hnf")
                        nc.scalar.activation(hnf[:], z2h[:], Act.Identity,
                                             bias=nmuh[:, 0:1], scale=istdh[:, 0:1])
                        if fl["gh"]:
                            nc.vector.tensor_mul(hnf[:], hnf[:], wt[f"gh_{l}"][:])
                        if fl["bh"]:
                            nc.vector.tensor_add(hnf[:], hnf[:], wt[f"bh_{l}"][:])
                        if l < L - 1:
                            nc.vector.tensor_copy(out=h_res[:, jsl], in_=hnf[:])
                        hn_src = hnf
                    else:
                        if l < L - 1:
                            nc.scalar.activation(h_res[:, jsl], z2h[:],
                                                 Act.Identity,
                                                 bias=nmuh[:, 0:1],
                                                 scale=istdh[:, 0:1])
                            hn_src = None
                        else:
                            hnf = sp.tile([P, d], f32, tag="hnf")
                            nc.scalar.activation(hnf[:], z2h[:], Act.Identity,
                                                 bias=nmuh[:, 0:1],
                                                 scale=istdh[:, 0:1])
                            hn_src = hnf

                    if l < L - 1:
                        src_ap = h_res[:, jsl] if hn_src is None else hn_src[:]
                        hnb = sp.tile([P, d], bf16, tag="hnb")
                        nc.vector.tensor_copy(out=hnb[:], in_=src_ap)
                        nc.sync.dma_start(out=h_new_bf[j * P:(j + 1) * P, :],
                                          in_=hnb[:])
                    else:
                        nc.sync.dma_start(out=h_out[j * P:(j + 1) * P, :],
                                          in_=hn_src[:])

                if l < L - 1:
                    nc.gpsimd.collective_compute(
                        "AllGather", mybir.AluOpType.bypass, replica_groups=rg,
                        ins=[h_new_bf[:]], outs=[h_full_bf[:]])

    nc.compile()
    return nc


# ---------------------------------------------------------------- entry
def kernel(H, E, ht, queries=None, **params):
    H = np.asarray(H, np.float32)
    E = np.asarray(E, np.float32)
    ht = np.asarray(ht)
    params = {k: np.asarray(v, np.float32) for k, v in params.items()}
    ncores = 8

    meta, per_core = _prep_host(H, E, ht, params, ncores)
    nc = _build_program(meta)

    from concourse.bass_utils import run_bass_kernel_spmd
    res = run_bass_kernel_spmd(nc, per_core, core_ids=list(range(ncores)))
    global LAST_EXEC_NS, LAST_NC, LAST_PER_CORE, LAST_META
    LAST_EXEC_NS = res.exec_time_ns
    LAST_NC = nc
    LAST_PER_CORE = per_core
    LAST_META = meta
    shards = [res.results[c]["h_out"] for c in range(ncores)]
    out = np.concatenate(shards, axis=0)[:meta["n"]]
    return np.ascontiguousarray(out.astype(np.float32))


LAST_EXEC_NS = None
LAST_NC = None
LAST_PER_CORE = None
LAST_META = None
